# revision 27
# baseline (speedup 1.0000x reference)
"""Trainium2 Bass kernel for nn_EnoughViT_63282048139394 — v5.

Single-token reduction (class-token chain, batch-broadcast output), plus:
  - ALL weights fp8 e3m4 with per-column scales (DMA 96MB -> 76MB = roofline)
  - u-direct GEMVs: LN scale folded into weights on host; every GEMV runs on
    the raw residual u while LN stats (mu, rstd) compute concurrently on the
    vector engine; rstd/mu corrections applied in a short cm epilogue.
    Exact algebra: h@W = r*[u@(diag(s)W) - mu*colsum(diag(s)W)] + b@W
  - attention GEMVs stream 4 concurrent column-strips
  - full-layer-ahead weight prefetch + classifier prefetch at layer 10
"""

import numpy as np
import ml_dtypes
from contextlib import ExitStack

import concourse.bass as bass
import concourse.tile as tile
from concourse import bacc, mybir
from concourse.bass_utils import run_bass_kernel_spmd

E = 768
HID = 3072
CLS = 1000
L = 12
EPS = 1e-5
INV_SQRT_E = 1.0 / float(np.sqrt(768.0))
DT = mybir.dt.float32
BF = mybir.dt.bfloat16
F8 = mybir.dt.float8e3
AX = mybir.AxisListType
OP = mybir.AluOpType
ACT = mybir.ActivationFunctionType
F8LIM = 12.0

# HAM warm-filler counts (N=512 bf16 junk, ~216ns each @2.4GHz), placed at
# the PE-queue points where the engine would otherwise idle on DVE glue
W_PRO = 14   # program start (ramp HAM while first weights stream)
W_A1 = 8     # after attn GEMV (evac wait)
W_A2 = 4     # after t transposes (t-epi)
W_A3 = 2     # after dot matmul (sval/c0)
W_A4 = 8     # after c0 bcast (a-chain + u' + cast)
W_M1 = 6     # after W1 GEMV (evac-A wait)
W_M2 = 6     # after W1 A-transposes (gfA + geluA)
W_M3 = 4     # after W2 GEMV (evac wait)
W_M4 = 6     # after W2 transposes (epi + residual + cast)

# pv column map (per layer, cm layout [128, 132]):
#  0:6  qv    6:12 qt   12:18 cv   18:24 ct   24:30 bv   30:36 bt
# 36:42 q2   42:48 b2   48:54 ln1_s  54:60 ln1_b
# 60:84 q1   84:108 c1  108:132 bb1
PV_W = 132


def build_program(gelu_mode='hw'):
    nc = bacc.Bacc()
    inp = {}

    def din(name, shape, dt=DT):
        t = nc.dram_tensor(name, list(shape), dt, kind="ExternalInput")
        inp[name] = t
        return t

    for l in range(L):
        din(f"wvt{l}", (128, 6 * 1536), F8)
        din(f"w1t{l}", (128, 6 * HID), F8)
        din(f"w2t{l}", (128, 6 * 4 * E), F8)
        din(f"pv{l}", (128, PV_W))
    din("wc1t", (128, 6 * HID), F8)
    for c in range(8):
        din(f"wc2{c}", (128, 3 * CLS), F8)
    din("fcm", (128, 72))          # qc1(24) | ccl(24) | bcl(24)
    din("fb", (1, CLS))            # bc2 flat
    din("qc2", (1, CLS))           # wc2 column scales, flat
    din("identf", (128, 128))
    din("onesc", (128, 1))
    din("onesr", (1, 128))
    din("u0", (128, 6))

    out_t = nc.dram_tensor("out", [1, CLS], DT, kind="ExternalOutput")

    with ExitStack() as ctx:
        tc = ctx.enter_context(tile.TileContext(nc))
        wsm = ctx.enter_context(tc.tile_pool(name="wsm", bufs=3))
        wbg = ctx.enter_context(tc.tile_pool(name="wbg", bufs=3))
        wclf = ctx.enter_context(tc.tile_pool(name="wclf", bufs=1))
        vp = ctx.enter_context(tc.tile_pool(name="vp", bufs=2))
        pers = ctx.enter_context(tc.tile_pool(name="pers", bufs=1))
        wk = ctx.enter_context(tc.tile_pool(name="wk", bufs=1))
        ps_at = ctx.enter_context(tc.tile_pool(name="ps_at", bufs=1, space="PSUM"))
        ps_m = ctx.enter_context(tc.tile_pool(name="ps_m", bufs=1, space="PSUM"))
        ps_t = ctx.enter_context(tc.tile_pool(name="ps_t", bufs=1, space="PSUM"))

        psS = ps_t.tile([128, 128], DT)

        ident = pers.tile([128, 128], DT)
        nc.sync.dma_start(out=ident[:], in_=inp["identf"][:, :])
        onesc = pers.tile([128, 1], DT)
        nc.sync.dma_start(out=onesc[:], in_=inp["onesc"][:, :])
        onesr = pers.tile([1, 128], DT)
        nc.sync.dma_start(out=onesr[:], in_=inp["onesr"][:, :])
        epst = pers.tile([1, 1], DT)
        nc.vector.memset(epst[:], EPS)
        junk = pers.tile([1, 8], DT)
        nc.vector.memset(junk[:], 0.5)
        junkw = pers.tile([128, 512], BF)
        nc.vector.memset(junkw[:], 0.0)
        junkc = pers.tile([128, 1], BF)
        nc.vector.memset(junkc[:], 0.0)
        psJ = ps_m.tile([128, 512], DT, tag="mJ")

        def warm(n):
            for _ in range(n):
                nc.tensor.matmul(psJ[0:1, 0:512], junkc[:], junkw[:],
                                 start=True, stop=True)

        stat_in = pers.tile([128, 12], DT)
        u_cm = stat_in[:, 0:6]
        nc.sync.dma_start(out=stat_in[:, 0:6], in_=inp["u0"][:, :])

        def gelu_to(out_bf, in_f32, shp):
            if gelu_mode == 'hw':
                nc.scalar.activation(out=out_bf[:], in_=in_f32[:], func=ACT.Gelu)
                return
            y = wk.tile(shp, DT, tag="gely")
            nc.vector.tensor_mul(y[:], in_f32[:], in_f32[:])
            nc.vector.tensor_scalar(
                out=y[:], in0=y[:], scalar1=0.044715, scalar2=1.0,
                op0=OP.mult, op1=OP.add)
            nc.vector.tensor_mul(y[:], y[:], in_f32[:])
            nc.scalar.activation(out=y[:], in_=y[:], func=ACT.Tanh,
                                 scale=float(np.sqrt(2.0 / np.pi)))
            nc.vector.tensor_scalar(
                out=y[:], in0=y[:], scalar1=1.0, scalar2=0.5,
                op0=OP.add, op1=OP.mult)
            nc.vector.tensor_mul(y[:], in_f32[:], y[:])
            nc.vector.tensor_copy(out=out_bf[:], in_=y[:])

        def dummy(fn, dep=None):
            if gelu_mode != 'hw' and fn == ACT.Gelu:
                fn = ACT.Tanh
            if dep is None:
                dep = junk[0:1, 0:4]
            w = dep.shape[1] if hasattr(dep, 'shape') else 4
            w = min(4, w)
            with tc.high_priority():
                nc.scalar.activation(out=junk[0:1, 4:4 + w], in_=dep[0:1, 0:w],
                                     func=fn, scale=0.0, bias=epst[:])

        def stats_mm(x_cm):
            """emit sq-mul (DVE) now; PE stats matmul goes in caller's order"""
            nc.vector.tensor_mul(stat_in[:, 6:12], x_cm, x_cm)

        def stats_finish(tag):
            """DVE/ACT chain -> scal3 [1,3] = (mu, r, -mu*r); runs while the
            tensor engine streams the GEMV."""
            scal2 = wk.tile([1, 3], DT, tag=tag + "sc")
            scr6 = wk.tile([1, 6], DT, tag=tag + "s6")
            nc.vector.tensor_scalar(
                out=scr6[:], in0=psS[0:1, 0:6], scalar1=1.0 / E, scalar2=None,
                op0=OP.mult, op1=OP.add, accum_out=scal2[0:1, 0:1])
            msq = wk.tile([1, 1], DT, tag=tag + "ms")
            nc.vector.tensor_scalar(
                out=scr6[:], in0=psS[0:1, 6:12], scalar1=1.0 / E, scalar2=None,
                op0=OP.mult, op1=OP.add, accum_out=msq[:])
            mu2 = wk.tile([1, 1], DT, tag=tag + "m2")
            nc.vector.tensor_mul(mu2[:], scal2[0:1, 0:1], scal2[0:1, 0:1])
            var = wk.tile([1, 1], DT, tag=tag + "va")
            nc.vector.tensor_scalar(
                out=var[:], in0=msq[:], scalar1=1.0, scalar2=mu2[:],
                op0=OP.mult, op1=OP.subtract)
            sd = wk.tile([1, 1], DT, tag=tag + "sd")
            nc.scalar.activation(out=sd[:], in_=var[:], func=ACT.Sqrt, bias=epst[:])
            nc.vector.reciprocal(scal2[0:1, 1:2], sd[:])
            nc.vector.tensor_scalar(
                out=scal2[0:1, 2:3], in0=scal2[0:1, 0:1],
                scalar1=scal2[0:1, 1:2], scalar2=-1.0,
                op0=OP.mult, op1=OP.mult)
            return scal2, sd

        def stats_bcast(scal2, tag):
            """PE bcast mm (caller controls queue position) + DVE copy"""
            pbc = psS[:, 12:15]
            nc.tensor.matmul(pbc, onesr[:], scal2[:], start=True, stop=True)
            bcs = wk.tile([128, 3], DT, tag=tag + "bc")
            nc.vector.tensor_copy(out=bcs[:], in_=pbc)
            return bcs[:, 0:1], bcs[:, 1:2], bcs[:, 2:3]  # mu, r, -mu*r

        def load_layer(l):
            pv = vp.tile([128, PV_W], DT, tag="pv", name=f"pv{l}_t")
            nc.sync.dma_start(out=pv[:], in_=inp[f"pv{l}"][:, :])
            wvt = wsm.tile([128, 6 * 1536], F8, tag="wvt", name=f"wvt{l}_t")
            nc.sync.dma_start(out=wvt[:], in_=inp[f"wvt{l}"][:, :])
            w1t = wbg.tile([128, 6 * HID], F8, tag="w1t", name=f"w1t{l}_t")
            nc.sync.dma_start(out=w1t[:], in_=inp[f"w1t{l}"][:, :])
            w2t = wbg.tile([128, 6 * 4 * E], F8, tag="w2t", name=f"w2t{l}_t")
            nc.sync.dma_start(out=w2t[:], in_=inp[f"w2t{l}"][:, :])
            return wvt, w1t, w2t, pv

        def load_classifier():
            fcm = vp.tile([128, 72], DT, tag="fcm")
            nc.sync.dma_start(out=fcm[:], in_=inp["fcm"][:, :])
            fb = pers.tile([1, CLS], DT)
            nc.sync.dma_start(out=fb[:], in_=inp["fb"][:, :])
            qc2 = pers.tile([1, CLS], DT)
            nc.sync.dma_start(out=qc2[:], in_=inp["qc2"][:, :])
            wc1t = wclf.tile([128, 6 * HID], F8, tag="wc1t")
            nc.sync.dma_start(out=wc1t[:], in_=inp["wc1t"][:, :])
            wc2_ = []
            for c in range(8):
                w = wclf.tile([128, 3 * CLS], F8, tag=f"wc2{c}")
                nc.sync.dma_start(out=w[:], in_=inp[f"wc2{c}"][:, :])
                wc2_.append(w)
            return fcm, fb, qc2, wc1t, wc2_

        warm(W_PRO)
        nxt = load_layer(0)
        clf = None
        u_bf = pers.tile([128, 6], BF)
        nc.vector.tensor_copy(out=u_bf[:], in_=u_cm)
        stats_mm(u_cm)

        for l in range(L):
            wvt, w1t, w2t, pv = nxt
            if l + 1 < L:
                nxt = load_layer(l + 1)
            if l == 9:
                clf = load_classifier()

            # ---- attn GEMV on raw u; LN1 stats overlap the streaming ----
            psA = ps_at.tile([128, 384], DT, tag="pa")
            for s in range(6):
                st, sp = (s == 0), (s == 5)
                lhs = u_bf[:, s:s + 1]
                for g in range(4):
                    nc.tensor.matmul(
                        psA[32 * g:32 * g + 1, 0:384], lhs,
                        wvt[:, (s * 4 + g) * 384:(s * 4 + g + 1) * 384],
                        start=st, stop=sp, tile_position=(0, 32 * g),
                        skip_group_check=True)
                if s == 0:
                    nc.tensor.matmul(psS[0:1, 0:12], onesc[:], stat_in[:],
                                     start=True, stop=True)
            scal3, _sd1 = stats_finish("l1")
            warm(3)
            nc.tensor.matmul(psS[:, 12:15], onesr[:], scal3[:],
                             start=True, stop=True)

            # evac first (t rows 0,32 land before a rows 64,96); precomputes
            # interleave on the V queue between the copies
            at_sb = wk.tile([128, 384], DT, tag="atsb")
            at_sb2 = wk.tile([66, 384], DT, tag="atsb2")
            nc.vector.tensor_copy(out=at_sb[0:1, :], in_=psA[0:1, :])
            nc.scalar.copy(out=at_sb[32:33, :], in_=psA[32:33, :])
            bcs = wk.tile([128, 3], DT, tag="l1bc2")
            nc.vector.tensor_copy(out=bcs[:], in_=psS[:, 12:15])
            mu_bc, r_bc, nmr_bc = bcs[:, 0:1], bcs[:, 1:2], bcs[:, 2:3]
            nc.vector.tensor_copy(out=at_sb[64:65, :], in_=psA[64:65, :])
            nc.scalar.copy(out=at_sb2[64:65, :], in_=psA[96:97, :])
            btm = wk.tile([128, 6], DT, tag="btm")
            nc.vector.scalar_tensor_tensor(
                out=btm[:], in0=pv[:, 18:24], scalar=nmr_bc, in1=pv[:, 30:36],
                op0=OP.mult, op1=OP.add)
            z = wk.tile([128, 6], DT, tag="z6")
            nc.vector.tensor_scalar(out=z[:], in0=u_cm, scalar1=mu_bc,
                                    scalar2=None, op0=OP.subtract)
            hf = wk.tile([128, 6], DT, tag="hf")
            nc.vector.tensor_mul(hf[:], z[:], pv[:, 48:54])
            nc.vector.scalar_tensor_tensor(
                out=hf[:], in0=hf[:], scalar=r_bc, in1=pv[:, 54:60],
                op0=OP.mult, op1=OP.add)
            bvm = wk.tile([128, 6], DT, tag="bvm")
            nc.vector.scalar_tensor_tensor(
                out=bvm[:], in0=pv[:, 12:18], scalar=nmr_bc, in1=pv[:, 24:30],
                op0=OP.mult, op1=OP.add)
            # transposes: t (rows 0,32) -> psS 16+seg; a (rows 64,96) -> 22+seg
            def at_transp(rlist):
                for r in rlist:
                    base = 16 if r < 2 else 22
                    srcr, part = (at_sb, 32 * r) if r < 3 else (at_sb2, 64)
                    idr = ident[part:part + 1, part:part + 1]
                    for c in range(3):
                        seg = 3 * (r % 2) + c
                        nc.tensor.transpose(
                            psS[:, base + seg:base + seg + 1],
                            srcr[part:part + 1, 128 * c:128 * c + 128], idr)
            at_transp([0, 1])
            warm(W_A2)
            at_transp([2, 3])

            # t_true = r*(T*qt) + btm ; sval = INV_SQRT_E * (h . t_true)
            tt = wk.tile([128, 6], DT, tag="tt6")
            nc.vector.tensor_mul(tt[:], psS[:, 16:22], pv[:, 6:12])
            nc.vector.scalar_tensor_tensor(
                out=tt[:], in0=tt[:], scalar=r_bc, in1=btm[:],
                op0=OP.mult, op1=OP.add)
            scr = wk.tile([128, 6], DT, tag="scr6b")
            nc.vector.tensor_mul(scr[:], hf[:], tt[:])
            pdd = psS[0:1, 28:34]
            nc.tensor.matmul(pdd, onesc[:], scr[:], start=True, stop=True)
            warm(W_A3)
            s6 = wk.tile([1, 6], DT, tag="sv6")
            sval = wk.tile([1, 1], DT, tag="sval")
            nc.vector.tensor_scalar(
                out=s6[:], in0=psS[0:1, 28:34], scalar1=INV_SQRT_E, scalar2=None,
                op0=OP.mult, op1=OP.add, accum_out=sval[:])
            c0 = wk.tile([1, 1], DT, tag="c0")
            nc.vector.tensor_scalar(
                out=c0[:], in0=sval[:], scalar1=1.0, scalar2=None, op0=OP.add)
            pb = psS[:, 15:16]
            nc.tensor.matmul(pb, onesr[:], c0[:], start=True, stop=True)
            c0b = wk.tile([128, 1], DT, tag="c0b")
            nc.vector.tensor_copy(out=c0b[:], in_=pb)
            # a_true = r*(A*qv) + bvm ; u' = h + c0*a_true
            aq = wk.tile([128, 6], DT, tag="aq6")
            nc.vector.tensor_mul(aq[:], psS[:, 22:28], pv[:, 0:6])
            nc.vector.scalar_tensor_tensor(
                out=aq[:], in0=aq[:], scalar=r_bc, in1=bvm[:],
                op0=OP.mult, op1=OP.add)
            nc.vector.scalar_tensor_tensor(
                out=u_cm, in0=aq[:], scalar=c0b[:, 0:1], in1=hf[:],
                op0=OP.mult, op1=OP.add)
            nc.vector.tensor_copy(out=u_bf[:], in_=u_cm)
            stats_mm(u_cm)
            warm(W_A4)

            # ---- W1 GEMV on raw u'; LN2 stats overlap ----
            psC = ps_m.tile([128, 512], DT, tag="mC")
            psD = ps_m.tile([128, 512], DT, tag="mD")
            for s in range(6):
                st, sp = (s == 0), (s == 5)
                lhs = u_bf[:, s:s + 1]
                for nt in range(6):
                    pt, row = (psC, nt) if nt < 4 else (psD, nt - 4)
                    nc.tensor.matmul(
                        pt[32 * row:32 * row + 1, 0:512], lhs,
                        w1t[:, s * HID + nt * 512: s * HID + nt * 512 + 512],
                        start=st, stop=sp, tile_position=(0, 32 * row),
                        skip_group_check=True)
                if s == 0:
                    nc.tensor.matmul(psS[0:1, 0:12], onesc[:], stat_in[:],
                                     start=True, stop=True)
            scal3b, sd2 = stats_finish("l2")
            dummy(ACT.Gelu, sd2)   # Gelu table load after the l2 sqrt, hidden
            warm(4)
            nc.tensor.matmul(psS[:, 12:15], onesr[:], scal3b[:],
                             start=True, stop=True)
            warm(W_M1)

            # evac half A (psC = hidden segs 0..15) first, V copies lead
            m1r = wk.tile([66, 512], DT, tag="m1r")
            m1r2 = wk.tile([66, 512], DT, tag="m1r2")
            nc.vector.tensor_copy(out=m1r[0:1, :], in_=psC[0:1, :])
            nc.scalar.copy(out=m1r[32:33, :], in_=psC[32:33, :])
            nc.vector.tensor_copy(out=m1r[64:65, :], in_=psC[64:65, :])
            nc.scalar.copy(out=m1r2[64:65, :], in_=psC[96:97, :])
            bcs2 = wk.tile([128, 3], DT, tag="l2bc2")
            nc.vector.tensor_copy(out=bcs2[:], in_=psS[:, 12:15])
            mu2_bc, r2_bc, nmr2_bc = bcs2[:, 0:1], bcs2[:, 1:2], bcs2[:, 2:3]
            bbm = wk.tile([128, 24], DT, tag="bbm")
            nc.vector.scalar_tensor_tensor(
                out=bbm[:], in0=pv[:, 84:108], scalar=nmr2_bc,
                in1=pv[:, 108:132], op0=OP.mult, op1=OP.add)
            upb = wk.tile([128, 6], DT, tag="upb")
            nc.vector.tensor_add(upb[:], u_cm, pv[:, 42:48])
            gps = psS[:, 34:58]
            for nt in range(4):
                srcr, base = (m1r, 32 * nt) if nt < 3 else (m1r2, 64)
                idr = ident[base:base + 1, base:base + 1]
                for c in range(4):
                    nc.tensor.transpose(
                        psS[:, 34 + 4 * nt + c:35 + 4 * nt + c],
                        srcr[base:base + 1, 128 * c:128 * c + 128], idr)
            warm(W_M2)
            # half A epilogue + gelu -> W2 can start streaming segs 0..15
            gf = wk.tile([128, 24], DT, tag="gf")
            nc.vector.tensor_mul(gf[:, 0:16], pv[:, 60:76], gps[:, 0:16])
            nc.vector.scalar_tensor_tensor(
                out=gf[:, 0:16], in0=gf[:, 0:16], scalar=r2_bc,
                in1=bbm[:, 0:16], op0=OP.mult, op1=OP.add)
            g_bf = wk.tile([128, 24], BF, tag="gbf")
            nc.scalar.activation(out=g_bf[:, 0:16], in_=gf[:, 0:16],
                                 func=ACT.Gelu)

            # ---- W2 GEMV: 24 passes x 2 chunks of 384; half B epilogue
            # (psD = segs 16..23) overlaps the first 16 waves ----
            psE = ps_m.tile([128, 384], DT, tag="mE")

            def w2_waves(lo, hi):
                for s in range(lo, hi):
                    st, sp = (s == 0), (s == 23)
                    lhs = g_bf[:, s:s + 1]
                    for g in range(2):
                        nc.tensor.matmul(
                            psE[32 * g:32 * g + 1, 0:384], lhs,
                            w2t[:, (s // 4) * 4 * E + (s % 4) * E + g * 384:
                                (s // 4) * 4 * E + (s % 4) * E + (g + 1) * 384],
                            start=st, stop=sp, tile_position=(0, 32 * g),
                            skip_group_check=True)

            w2_waves(0, 16)
            nc.vector.tensor_copy(out=m1r2[0:1, :], in_=psD[0:1, :])
            nc.scalar.copy(out=m1r2[32:33, :], in_=psD[32:33, :])
            for nt in range(4, 6):
                srcr, base = m1r2, 32 * (nt - 4)
                idr = ident[base:base + 1, base:base + 1]
                for c in range(4):
                    nc.tensor.transpose(
                        psS[:, 34 + 4 * nt + c:35 + 4 * nt + c],
                        srcr[base:base + 1, 128 * c:128 * c + 128], idr)
            nc.vector.tensor_mul(gf[:, 16:24], pv[:, 76:84], gps[:, 16:24])
            nc.vector.scalar_tensor_tensor(
                out=gf[:, 16:24], in0=gf[:, 16:24], scalar=r2_bc,
                in1=bbm[:, 16:24], op0=OP.mult, op1=OP.add)
            nc.scalar.activation(out=g_bf[:, 16:24], in_=gf[:, 16:24],
                                 func=ACT.Gelu)
            dummy(ACT.Sqrt, g_bf[0:1, 0:4])
            w2_waves(16, 24)
            warm(W_M3)
            m2r = wk.tile([34, 384], DT, tag="m2r")
            nc.vector.tensor_copy(out=m2r[0:1, :], in_=psE[0:1, :])
            nc.scalar.copy(out=m2r[32:33, :], in_=psE[32:33, :])
            pu2 = psS[:, 58:64]
            for r in range(2):
                idr = ident[32 * r:32 * r + 1, 32 * r:32 * r + 1]
                for c in range(3):
                    nc.tensor.transpose(
                        psS[:, 58 + 3 * r + c:59 + 3 * r + c],
                        m2r[32 * r:32 * r + 1, 128 * c:128 * c + 128], idr)
            warm(W_M4)
            # u'' = (u' + b2) + m2q*q2
            d6 = wk.tile([128, 6], DT, tag="d6")
            nc.vector.tensor_mul(d6[:], pv[:, 36:42], pu2)
            nc.vector.tensor_add(u_cm, upb[:], d6[:])
            nc.vector.tensor_copy(out=u_bf[:], in_=u_cm)
            stats_mm(u_cm)

        # ---- classifier (weights prefetched during layer 10) ----
        fcm, fb, qc2, wc1t, wc2 = clf

        stats_mm(u_cm)
        nc.tensor.matmul(psS[0:1, 0:12], onesc[:], stat_in[:],
                         start=True, stop=True)
        psC = ps_m.tile([128, 512], DT, tag="mC")
        psD = ps_m.tile([128, 512], DT, tag="mD")
        for s in range(6):
            st, sp = (s == 0), (s == 5)
            lhs = u_bf[:, s:s + 1]
            for nt in range(6):
                pt, row = (psC, nt) if nt < 4 else (psD, nt - 4)
                nc.tensor.matmul(
                    pt[32 * row:32 * row + 1, 0:512], lhs,
                    wc1t[:, s * HID + nt * 512: s * HID + nt * 512 + 512],
                    start=st, stop=sp, tile_position=(0, 32 * row),
                    skip_group_check=True)
        scal3f, sdf = stats_finish("lf")
        muf_bc, rf_bc, nmrf_bc = stats_bcast(scal3f, "lf")
        dummy(ACT.Gelu, sdf)
        bcmf = wk.tile([128, 24], DT, tag="bcmf")
        nc.vector.scalar_tensor_tensor(
            out=bcmf[:], in0=fcm[:, 24:48], scalar=nmrf_bc, in1=fcm[:, 48:72],
            op0=OP.mult, op1=OP.add)
        warm(6)
        m1r = wk.tile([66, 512], DT, tag="m1r")
        nc.vector.tensor_copy(out=m1r[0:1, :], in_=psC[0:1, :])
        nc.scalar.copy(out=m1r[32:33, :], in_=psC[32:33, :])
        nc.vector.tensor_copy(out=m1r[64:65, :], in_=psC[64:65, :])
        m1r2 = wk.tile([66, 512], DT, tag="m1r2")
        nc.scalar.copy(out=m1r2[64:65, :], in_=psC[96:97, :])
        nc.vector.tensor_copy(out=m1r2[0:1, :], in_=psD[0:1, :])
        nc.scalar.copy(out=m1r2[32:33, :], in_=psD[32:33, :])
        gps = psS[:, 34:58]
        for nt in range(6):
            if nt < 3:
                srcr, base = m1r, 32 * nt
            elif nt == 3:
                srcr, base = m1r2, 64
            else:
                srcr, base = m1r2, 32 * (nt - 4)
            idr = ident[base:base + 1, base:base + 1]
            for c in range(4):
                nc.tensor.transpose(
                    psS[:, 34 + 4 * nt + c:35 + 4 * nt + c],
                    srcr[base:base + 1, 128 * c:128 * c + 128], idr)
        warm(3)
        gf = wk.tile([128, 24], DT, tag="gf")
        nc.vector.tensor_mul(gf[:], gps, fcm[:, 0:24])
        nc.vector.scalar_tensor_tensor(
            out=gf[:], in0=gf[:], scalar=rf_bc, in1=bcmf[:],
            op0=OP.mult, op1=OP.add)
        gc_bf = wk.tile([128, 24], BF, tag="gbf")
        gelu_to(gc_bf, gf, [128, 24])

        psF = ps_m.tile([128, 512], DT, tag="mF")
        for s in range(24):
            st, sp = (s == 0), (s == 23)
            lhs = gc_bf[:, s:s + 1]
            wsrc = wc2[s // 3]
            sl = s % 3
            for g in range(2):
                nc.tensor.matmul(
                    psF[32 * g:32 * g + 1, 0:500], lhs,
                    wsrc[:, sl * CLS + g * 500: sl * CLS + (g + 1) * 500],
                    start=st, stop=sp, tile_position=(0, 32 * g),
                    skip_group_check=True)
        warm(2)
        lg = wk.tile([1, CLS], DT, tag="lg")
        nc.vector.tensor_copy(out=lg[0:1, 0:500], in_=psF[0:1, 0:500])
        nc.scalar.copy(out=lg[0:1, 500:1000], in_=psF[32:33, 0:500])
        nc.vector.tensor_mul(lg[:], lg[:], qc2[:])
        nc.vector.tensor_add(lg[:], lg[:], fb[:])

        # log_softmax
        mx = wk.tile([1, 1], DT, tag="mx")
        nc.vector.reduce_max(mx[:], lg[:], axis=AX.X)
        sh = wk.tile([1, CLS], DT, tag="sh")
        nc.vector.tensor_scalar(
            out=sh[:], in0=lg[:], scalar1=mx[:], scalar2=None, op0=OP.subtract)
        se = wk.tile([1, 1], DT, tag="se")
        nc.scalar.activation(out=lg[:], in_=sh[:], func=ACT.Exp, accum_out=se[:])
        lse = wk.tile([1, 1], DT, tag="lse")
        nc.scalar.activation(out=lse[:], in_=se[:], func=ACT.Ln)
        nc.vector.tensor_scalar(
            out=sh[:], in0=sh[:], scalar1=lse[:], scalar2=None, op0=OP.subtract)
        nc.sync.dma_start(out=out_t[:, :], in_=sh[:])

    nc.compile()
    return nc


def _cm(v, nseg):
    """flat [-1] -> [128, nseg] with cm[p, s] = v[128s + p]"""
    return np.ascontiguousarray(np.asarray(v, np.float32).reshape(nseg, 128).T)


def _q8(w):
    """[K, N] f32 -> (fp8 e3m4 col-scaled, scales[N])"""
    w = np.asarray(w, np.float32)
    s = np.abs(w).max(axis=0) / F8LIM
    s = np.where(s == 0, 1.0, s)
    return (w / s).astype(ml_dtypes.float8_e3m4), s


def prep_inputs(inputs):
    f32 = lambda x: np.ascontiguousarray(np.asarray(x, dtype=np.float32))
    m = {}
    Wv, Wt = inputs["Wv"], inputs["Wtheta"]
    W1, W2 = inputs["W1"], inputs["W2"]
    for l in range(L):
        s1l = np.asarray(inputs["ln1_s"][l], np.float32)
        b1l = np.asarray(inputs["ln1_b"][l], np.float32)
        s2l = np.asarray(inputs["ln2_s"][l], np.float32)
        b2l = np.asarray(inputs["ln2_b"][l], np.float32)
        wv = np.asarray(Wv[l], np.float32) * s1l[:, None]    # diag(s)Wv
        wt = np.asarray(Wt[l], np.float32) * s1l[:, None]
        cv = wv.sum(axis=0)
        ct = wt.sum(axis=0)
        bv = b1l @ np.asarray(Wv[l], np.float32)
        bt = b1l @ np.asarray(Wt[l], np.float32)
        wvq, sv = _q8(wv)
        wtq, st = _q8(wt)
        wv3 = wvq.reshape(6, 128, 2, 384)
        wt3 = wtq.reshape(6, 128, 2, 384)
        blk = np.concatenate([wt3, wv3], axis=2)     # [6,128,4,384] t|t|a|a
        m[f"wvt{l}"] = np.ascontiguousarray(
            blk.transpose(1, 0, 2, 3).reshape(128, 6 * 1536))
        w1 = np.asarray(W1[l], np.float32) * s2l[:, None]    # diag(s2)W1
        c1 = w1.sum(axis=0)
        bb1 = b2l @ np.asarray(W1[l], np.float32) + np.asarray(
            inputs["b1"][l], np.float32)
        w1q, q1 = _q8(w1)
        m[f"w1t{l}"] = np.ascontiguousarray(
            w1q.reshape(6, 128, HID).transpose(1, 0, 2).reshape(128, 6 * HID))
        w2q, q2 = _q8(W2[l])
        m[f"w2t{l}"] = np.ascontiguousarray(
            w2q.reshape(6, 4, 128, E).transpose(2, 0, 1, 3).reshape(
                128, 6 * 4 * E))
        pv = np.concatenate([
            _cm(sv, 6), _cm(st, 6), _cm(cv, 6), _cm(ct, 6),
            _cm(bv, 6), _cm(bt, 6),
            _cm(q2, 6), _cm(inputs["b2"][l], 6),
            _cm(s1l, 6), _cm(b1l, 6),
            _cm(q1, 24), _cm(c1, 24), _cm(bb1, 24)], axis=1)
        m[f"pv{l}"] = f32(pv)
    sfl = np.asarray(inputs["lnf_s"], np.float32)
    bfl = np.asarray(inputs["lnf_b"], np.float32)
    wc1 = np.asarray(inputs["Wc1"], np.float32) * sfl[:, None]
    ccl = wc1.sum(axis=0)
    bcl = bfl @ np.asarray(inputs["Wc1"], np.float32) + np.asarray(
        inputs["bc1"], np.float32)
    wc1q, qc1 = _q8(wc1)
    m["wc1t"] = np.ascontiguousarray(
        wc1q.reshape(6, 128, HID).transpose(1, 0, 2).reshape(128, 6 * HID))
    wc2q, sc2 = _q8(inputs["Wc2"])
    wc2q = wc2q.reshape(24, 128, CLS).transpose(1, 0, 2)
    for c in range(8):
        m[f"wc2{c}"] = np.ascontiguousarray(
            wc2q[:, 3 * c:3 * c + 3].reshape(128, 3 * CLS))
    m["fcm"] = f32(np.concatenate([
        _cm(qc1, 24), _cm(ccl, 24), _cm(bcl, 24)], axis=1))
    m["fb"] = f32(np.asarray(inputs["bc2"]).reshape(1, CLS))
    m["qc2"] = f32(sc2.reshape(1, CLS))
    m["identf"] = np.eye(128, dtype=np.float32)
    m["onesc"] = np.ones((128, 1), np.float32)
    m["onesr"] = np.ones((1, 128), np.float32)
    u0 = np.asarray(inputs["class_token"]).reshape(E) + \
        np.asarray(inputs["pos"]).reshape(-1, E)[-1]
    m["u0"] = _cm(u0, 6)
    return m


_CACHED = {}


def kernel(**inputs) -> np.ndarray:
    b = int(np.asarray(inputs["x"]).shape[0])
    in_map = prep_inputs(inputs)
    if "nc" not in _CACHED:
        _CACHED["nc"] = build_program()
    nc = _CACHED["nc"]
    r = run_bass_kernel_spmd(nc, [in_map], core_ids=[0])
    out = np.asarray(r.results[0]["out"]).reshape(1, CLS)
    return np.ascontiguousarray(np.broadcast_to(out, (b, CLS)).astype(np.float32))


# revision 33
# speedup vs baseline: 1.0056x; 1.0056x over previous
"""Trainium2 Bass kernel for nn_EnoughViT_63282048139394 — v5.

Single-token reduction (class-token chain, batch-broadcast output), plus:
  - ALL weights fp8 e3m4 with per-column scales (DMA 96MB -> 76MB = roofline)
  - u-direct GEMVs: LN scale folded into weights on host; every GEMV runs on
    the raw residual u while LN stats (mu, rstd) compute concurrently on the
    vector engine; rstd/mu corrections applied in a short cm epilogue.
    Exact algebra: h@W = r*[u@(diag(s)W) - mu*colsum(diag(s)W)] + b@W
  - attention GEMVs stream 4 concurrent column-strips
  - full-layer-ahead weight prefetch + classifier prefetch at layer 10
"""

import numpy as np
import ml_dtypes
from contextlib import ExitStack

import concourse.bass as bass
import concourse.tile as tile
from concourse import bacc, mybir
from concourse.bass_utils import run_bass_kernel_spmd

E = 768
HID = 3072
CLS = 1000
L = 12
EPS = 1e-5
INV_SQRT_E = 1.0 / float(np.sqrt(768.0))
DT = mybir.dt.float32
BF = mybir.dt.bfloat16
F8 = mybir.dt.float8e3
AX = mybir.AxisListType
OP = mybir.AluOpType
ACT = mybir.ActivationFunctionType
F8LIM = 12.0

# HAM warm-filler counts (N=512 bf16 junk, ~216ns each @2.4GHz), placed at
# the PE-queue points where the engine would otherwise idle on DVE glue
W_PRO = 14   # program start (ramp HAM while first weights stream)
W_A1 = 8     # after attn GEMV (evac wait)
W_A2 = 4     # after t transposes (t-epi)
W_A3 = 2     # after dot matmul (sval/c0)
W_A4 = 8     # after c0 bcast (a-chain + u' + cast)
W_M1 = 10    # after W1 GEMV (evac-A wait)
W_M2 = 6     # after W1 A-transposes (gfA + geluA)
W_M3 = 6     # after W2 GEMV (evac wait)
W_M4 = 6     # after W2 transposes (epi + residual + cast)

# pv column map (per layer, cm layout [128, 132]):
#  0:6  qv    6:12 qt   12:18 cv   18:24 ct   24:30 bv   30:36 bt
# 36:42 q2   42:48 b2   48:54 ln1_s  54:60 ln1_b
# 60:84 q1   84:108 c1  108:132 bb1
PV_W = 132


def build_program(gelu_mode='hw'):
    nc = bacc.Bacc()
    inp = {}

    def din(name, shape, dt=DT):
        t = nc.dram_tensor(name, list(shape), dt, kind="ExternalInput")
        inp[name] = t
        return t

    for l in range(L):
        din(f"wvt{l}", (128, 6 * 1536), F8)
        din(f"w1t{l}", (128, 6 * HID), F8)
        din(f"w2t{l}", (128, 6 * 4 * E), F8)
        din(f"pv{l}", (128, PV_W))
    din("wc1t", (128, 6 * HID), F8)
    for c in range(8):
        din(f"wc2{c}", (128, 3 * CLS), F8)
    din("fcm", (128, 72))          # qc1(24) | ccl(24) | bcl(24)
    din("fb", (1, CLS))            # bc2 flat
    din("qc2", (1, CLS))           # wc2 column scales, flat
    din("identf", (128, 128))
    din("onesc", (128, 1))
    din("onesr", (1, 128))
    din("u0", (128, 6))

    out_t = nc.dram_tensor("out", [1, CLS], DT, kind="ExternalOutput")

    with ExitStack() as ctx:
        tc = ctx.enter_context(tile.TileContext(nc))
        wsm = ctx.enter_context(tc.tile_pool(name="wsm", bufs=3))
        wbg = ctx.enter_context(tc.tile_pool(name="wbg", bufs=3))
        wclf = ctx.enter_context(tc.tile_pool(name="wclf", bufs=1))
        vp = ctx.enter_context(tc.tile_pool(name="vp", bufs=2))
        pers = ctx.enter_context(tc.tile_pool(name="pers", bufs=1))
        wk = ctx.enter_context(tc.tile_pool(name="wk", bufs=1))
        ps_at = ctx.enter_context(tc.tile_pool(name="ps_at", bufs=1, space="PSUM"))
        ps_m = ctx.enter_context(tc.tile_pool(name="ps_m", bufs=1, space="PSUM"))
        ps_t = ctx.enter_context(tc.tile_pool(name="ps_t", bufs=1, space="PSUM"))

        psS = ps_t.tile([128, 128], DT)

        ident = pers.tile([128, 128], DT)
        nc.sync.dma_start(out=ident[:], in_=inp["identf"][:, :])
        onesc = pers.tile([128, 1], DT)
        nc.sync.dma_start(out=onesc[:], in_=inp["onesc"][:, :])
        onesr = pers.tile([1, 128], DT)
        nc.sync.dma_start(out=onesr[:], in_=inp["onesr"][:, :])
        epst = pers.tile([1, 1], DT)
        nc.vector.memset(epst[:], EPS)
        junk = pers.tile([1, 8], DT)
        nc.vector.memset(junk[:], 0.5)
        junkw = pers.tile([128, 512], BF)
        nc.vector.memset(junkw[:], 0.0)
        junkc = pers.tile([128, 1], BF)
        nc.vector.memset(junkc[:], 0.0)
        psJ = ps_m.tile([128, 512], DT, tag="mJ")

        def warm(n):
            for _ in range(n):
                nc.tensor.matmul(psJ[0:1, 0:512], junkc[:], junkw[:],
                                 start=True, stop=True)

        stat_in = pers.tile([128, 12], DT)
        u_cm = stat_in[:, 0:6]
        nc.sync.dma_start(out=stat_in[:, 0:6], in_=inp["u0"][:, :])

        def gelu_to(out_bf, in_f32, shp):
            if gelu_mode == 'hw':
                nc.scalar.activation(out=out_bf[:], in_=in_f32[:], func=ACT.Gelu)
                return
            y = wk.tile(shp, DT, tag="gely")
            nc.vector.tensor_mul(y[:], in_f32[:], in_f32[:])
            nc.vector.tensor_scalar(
                out=y[:], in0=y[:], scalar1=0.044715, scalar2=1.0,
                op0=OP.mult, op1=OP.add)
            nc.vector.tensor_mul(y[:], y[:], in_f32[:])
            nc.scalar.activation(out=y[:], in_=y[:], func=ACT.Tanh,
                                 scale=float(np.sqrt(2.0 / np.pi)))
            nc.vector.tensor_scalar(
                out=y[:], in0=y[:], scalar1=1.0, scalar2=0.5,
                op0=OP.add, op1=OP.mult)
            nc.vector.tensor_mul(y[:], in_f32[:], y[:])
            nc.vector.tensor_copy(out=out_bf[:], in_=y[:])

        def dummy(fn, dep=None):
            if gelu_mode != 'hw' and fn == ACT.Gelu:
                fn = ACT.Tanh
            if dep is None:
                dep = junk[0:1, 0:4]
            w = dep.shape[1] if hasattr(dep, 'shape') else 4
            w = min(4, w)
            with tc.high_priority():
                nc.scalar.activation(out=junk[0:1, 4:4 + w], in_=dep[0:1, 0:w],
                                     func=fn, scale=0.0, bias=epst[:])

        def stats_mm(x_cm):
            """emit sq-mul (DVE) now; PE stats matmul goes in caller's order"""
            nc.vector.tensor_mul(stat_in[:, 6:12], x_cm, x_cm)

        def stats_finish(tag):
            """DVE/ACT chain -> scal3 [1,3] = (mu, r, -mu*r); runs while the
            tensor engine streams the GEMV."""
            scal2 = wk.tile([1, 3], DT, tag=tag + "sc")
            scr6 = wk.tile([1, 6], DT, tag=tag + "s6")
            nc.vector.tensor_scalar(
                out=scr6[:], in0=psS[0:1, 0:6], scalar1=1.0 / E, scalar2=None,
                op0=OP.mult, op1=OP.add, accum_out=scal2[0:1, 0:1])
            msq = wk.tile([1, 1], DT, tag=tag + "ms")
            nc.vector.tensor_scalar(
                out=scr6[:], in0=psS[0:1, 6:12], scalar1=1.0 / E, scalar2=None,
                op0=OP.mult, op1=OP.add, accum_out=msq[:])
            mu2 = wk.tile([1, 1], DT, tag=tag + "m2")
            nc.vector.tensor_mul(mu2[:], scal2[0:1, 0:1], scal2[0:1, 0:1])
            var = wk.tile([1, 1], DT, tag=tag + "va")
            nc.vector.tensor_scalar(
                out=var[:], in0=msq[:], scalar1=1.0, scalar2=mu2[:],
                op0=OP.mult, op1=OP.subtract)
            sd = wk.tile([1, 1], DT, tag=tag + "sd")
            nc.scalar.activation(out=sd[:], in_=var[:], func=ACT.Sqrt, bias=epst[:])
            nc.vector.reciprocal(scal2[0:1, 1:2], sd[:])
            nc.vector.tensor_scalar(
                out=scal2[0:1, 2:3], in0=scal2[0:1, 0:1],
                scalar1=scal2[0:1, 1:2], scalar2=-1.0,
                op0=OP.mult, op1=OP.mult)
            return scal2, sd

        def stats_bcast(scal2, tag):
            """PE bcast mm (caller controls queue position) + DVE copy"""
            pbc = psS[:, 12:15]
            nc.tensor.matmul(pbc, onesr[:], scal2[:], start=True, stop=True)
            bcs = wk.tile([128, 3], DT, tag=tag + "bc")
            nc.vector.tensor_copy(out=bcs[:], in_=pbc)
            return bcs[:, 0:1], bcs[:, 1:2], bcs[:, 2:3]  # mu, r, -mu*r

        def load_layer(l):
            pv = vp.tile([128, PV_W], DT, tag="pv", name=f"pv{l}_t")
            nc.sync.dma_start(out=pv[:], in_=inp[f"pv{l}"][:, :])
            wvt = wsm.tile([128, 6 * 1536], F8, tag="wvt", name=f"wvt{l}_t")
            nc.sync.dma_start(out=wvt[:], in_=inp[f"wvt{l}"][:, :])
            w1t = wbg.tile([128, 6 * HID], F8, tag="w1t", name=f"w1t{l}_t")
            nc.sync.dma_start(out=w1t[:], in_=inp[f"w1t{l}"][:, :])
            w2t = wbg.tile([128, 6 * 4 * E], F8, tag="w2t", name=f"w2t{l}_t")
            nc.sync.dma_start(out=w2t[:], in_=inp[f"w2t{l}"][:, :])
            return wvt, w1t, w2t, pv

        def load_classifier():
            fcm = vp.tile([128, 72], DT, tag="fcm")
            nc.sync.dma_start(out=fcm[:], in_=inp["fcm"][:, :])
            fb = pers.tile([1, CLS], DT)
            nc.sync.dma_start(out=fb[:], in_=inp["fb"][:, :])
            qc2 = pers.tile([1, CLS], DT)
            nc.sync.dma_start(out=qc2[:], in_=inp["qc2"][:, :])
            wc1t = wclf.tile([128, 6 * HID], F8, tag="wc1t")
            nc.sync.dma_start(out=wc1t[:], in_=inp["wc1t"][:, :])
            wc2_ = []
            for c in range(8):
                w = wclf.tile([128, 3 * CLS], F8, tag=f"wc2{c}")
                nc.sync.dma_start(out=w[:], in_=inp[f"wc2{c}"][:, :])
                wc2_.append(w)
            return fcm, fb, qc2, wc1t, wc2_

        warm(W_PRO)
        nxt = load_layer(0)
        clf = None
        u_bf = pers.tile([128, 6], BF)
        nc.vector.tensor_copy(out=u_bf[:], in_=u_cm)
        stats_mm(u_cm)

        for l in range(L):
            wvt, w1t, w2t, pv = nxt
            if l + 1 < L:
                nxt = load_layer(l + 1)
            if l == 9:
                clf = load_classifier()

            # ---- attn GEMV on raw u; LN1 stats overlap the streaming ----
            psA = ps_at.tile([128, 384], DT, tag="pa")
            for s in range(6):
                st, sp = (s == 0), (s == 5)
                lhs = u_bf[:, s:s + 1]
                for g in range(4):
                    nc.tensor.matmul(
                        psA[32 * g:32 * g + 1, 0:384], lhs,
                        wvt[:, (s * 4 + g) * 384:(s * 4 + g + 1) * 384],
                        start=st, stop=sp, tile_position=(0, 32 * g),
                        skip_group_check=True)
                if s == 0:
                    nc.tensor.matmul(psS[0:1, 0:12], onesc[:], stat_in[:],
                                     start=True, stop=True)
            scal3, _sd1 = stats_finish("l1")
            nc.tensor.matmul(psS[:, 12:15], onesr[:], scal3[:],
                             start=True, stop=True)
            warm(W_A1)

            # evac first (t rows 0,32 land before a rows 64,96); precomputes
            # interleave on the V queue between the copies
            at_sb = wk.tile([128, 384], DT, tag="atsb")
            at_sb2 = wk.tile([66, 384], DT, tag="atsb2")
            nc.vector.tensor_copy(out=at_sb[0:1, :], in_=psA[0:1, :])
            nc.scalar.copy(out=at_sb[32:33, :], in_=psA[32:33, :])
            bcs = wk.tile([128, 3], DT, tag="l1bc2")
            nc.vector.tensor_copy(out=bcs[:], in_=psS[:, 12:15])
            mu_bc, r_bc, nmr_bc = bcs[:, 0:1], bcs[:, 1:2], bcs[:, 2:3]
            btm = wk.tile([128, 6], DT, tag="btm")
            nc.vector.scalar_tensor_tensor(
                out=btm[:], in0=pv[:, 18:24], scalar=nmr_bc, in1=pv[:, 30:36],
                op0=OP.mult, op1=OP.add)
            nc.vector.tensor_copy(out=at_sb[64:65, :], in_=psA[64:65, :])
            nc.scalar.copy(out=at_sb2[64:65, :], in_=psA[96:97, :])
            z = wk.tile([128, 6], DT, tag="z6")
            nc.vector.tensor_scalar(out=z[:], in0=u_cm, scalar1=mu_bc,
                                    scalar2=None, op0=OP.subtract)
            hf = wk.tile([128, 6], DT, tag="hf")
            nc.vector.tensor_mul(hf[:], z[:], pv[:, 48:54])
            nc.vector.scalar_tensor_tensor(
                out=hf[:], in0=hf[:], scalar=r_bc, in1=pv[:, 54:60],
                op0=OP.mult, op1=OP.add)
            bvm = wk.tile([128, 6], DT, tag="bvm")
            nc.vector.scalar_tensor_tensor(
                out=bvm[:], in0=pv[:, 12:18], scalar=nmr_bc, in1=pv[:, 24:30],
                op0=OP.mult, op1=OP.add)
            # transposes: t (rows 0,32) -> psS 16+seg; a (rows 64,96) -> 22+seg
            for r in range(4):
                base = 16 if r < 2 else 22
                srcr, part = (at_sb, 32 * r) if r < 3 else (at_sb2, 64)
                idr = ident[part:part + 1, part:part + 1]
                for c in range(3):
                    seg = 3 * (r % 2) + c
                    nc.tensor.transpose(
                        psS[:, base + seg:base + seg + 1],
                        srcr[part:part + 1, 128 * c:128 * c + 128], idr)
            warm(W_A2)

            # t_true = r*(T*qt) + btm ; sval = INV_SQRT_E * (h . t_true)
            tt = wk.tile([128, 6], DT, tag="tt6")
            nc.vector.tensor_mul(tt[:], psS[:, 16:22], pv[:, 6:12])
            nc.vector.scalar_tensor_tensor(
                out=tt[:], in0=tt[:], scalar=r_bc, in1=btm[:],
                op0=OP.mult, op1=OP.add)
            scr = wk.tile([128, 6], DT, tag="scr6b")
            nc.vector.tensor_mul(scr[:], hf[:], tt[:])
            pdd = psS[0:1, 28:34]
            nc.tensor.matmul(pdd, onesc[:], scr[:], start=True, stop=True)
            warm(W_A3)
            s6 = wk.tile([1, 6], DT, tag="sv6")
            sval = wk.tile([1, 1], DT, tag="sval")
            nc.vector.tensor_scalar(
                out=s6[:], in0=psS[0:1, 28:34], scalar1=INV_SQRT_E, scalar2=None,
                op0=OP.mult, op1=OP.add, accum_out=sval[:])
            c0 = wk.tile([1, 1], DT, tag="c0")
            nc.vector.tensor_scalar(
                out=c0[:], in0=sval[:], scalar1=1.0, scalar2=None, op0=OP.add)
            pb = psS[:, 15:16]
            nc.tensor.matmul(pb, onesr[:], c0[:], start=True, stop=True)
            c0b = wk.tile([128, 1], DT, tag="c0b")
            nc.vector.tensor_copy(out=c0b[:], in_=pb)
            # a_true = r*(A*qv) + bvm ; u' = h + c0*a_true
            aq = wk.tile([128, 6], DT, tag="aq6")
            nc.vector.tensor_mul(aq[:], psS[:, 22:28], pv[:, 0:6])
            nc.vector.scalar_tensor_tensor(
                out=aq[:], in0=aq[:], scalar=r_bc, in1=bvm[:],
                op0=OP.mult, op1=OP.add)
            nc.vector.scalar_tensor_tensor(
                out=u_cm, in0=aq[:], scalar=c0b[:, 0:1], in1=hf[:],
                op0=OP.mult, op1=OP.add)
            nc.vector.tensor_copy(out=u_bf[:], in_=u_cm)
            stats_mm(u_cm)
            warm(W_A4)

            # ---- W1 GEMV on raw u'; LN2 stats overlap ----
            psC = ps_m.tile([128, 512], DT, tag="mC")
            psD = ps_m.tile([128, 512], DT, tag="mD")
            for s in range(6):
                st, sp = (s == 0), (s == 5)
                lhs = u_bf[:, s:s + 1]
                for nt in range(6):
                    pt, row = (psC, nt) if nt < 4 else (psD, nt - 4)
                    nc.tensor.matmul(
                        pt[32 * row:32 * row + 1, 0:512], lhs,
                        w1t[:, s * HID + nt * 512: s * HID + nt * 512 + 512],
                        start=st, stop=sp, tile_position=(0, 32 * row),
                        skip_group_check=True)
                if s == 0:
                    nc.tensor.matmul(psS[0:1, 0:12], onesc[:], stat_in[:],
                                     start=True, stop=True)
            scal3b, sd2 = stats_finish("l2")
            dummy(ACT.Gelu, sd2)   # Gelu table load after the l2 sqrt, hidden
            nc.tensor.matmul(psS[:, 12:15], onesr[:], scal3b[:],
                             start=True, stop=True)
            warm(W_M1)

            # evac half A (psC = hidden segs 0..15) first, V copies lead
            m1r = wk.tile([66, 512], DT, tag="m1r")
            m1r2 = wk.tile([66, 512], DT, tag="m1r2")
            nc.vector.tensor_copy(out=m1r[0:1, :], in_=psC[0:1, :])
            nc.scalar.copy(out=m1r[32:33, :], in_=psC[32:33, :])
            bcs2 = wk.tile([128, 3], DT, tag="l2bc2")
            nc.vector.tensor_copy(out=bcs2[:], in_=psS[:, 12:15])
            mu2_bc, r2_bc, nmr2_bc = bcs2[:, 0:1], bcs2[:, 1:2], bcs2[:, 2:3]
            bbm = wk.tile([128, 24], DT, tag="bbm")
            nc.vector.scalar_tensor_tensor(
                out=bbm[:], in0=pv[:, 84:108], scalar=nmr2_bc,
                in1=pv[:, 108:132], op0=OP.mult, op1=OP.add)
            nc.vector.tensor_copy(out=m1r[64:65, :], in_=psC[64:65, :])
            nc.scalar.copy(out=m1r2[64:65, :], in_=psC[96:97, :])
            upb = wk.tile([128, 6], DT, tag="upb")
            nc.vector.tensor_add(upb[:], u_cm, pv[:, 42:48])
            gps = psS[:, 34:58]
            for nt in range(4):
                srcr, base = (m1r, 32 * nt) if nt < 3 else (m1r2, 64)
                idr = ident[base:base + 1, base:base + 1]
                for c in range(4):
                    nc.tensor.transpose(
                        psS[:, 34 + 4 * nt + c:35 + 4 * nt + c],
                        srcr[base:base + 1, 128 * c:128 * c + 128], idr)
            warm(W_M2)
            # half A epilogue + gelu -> W2 can start streaming segs 0..15
            gf = wk.tile([128, 24], DT, tag="gf")
            nc.vector.tensor_mul(gf[:, 0:16], pv[:, 60:76], gps[:, 0:16])
            nc.vector.scalar_tensor_tensor(
                out=gf[:, 0:16], in0=gf[:, 0:16], scalar=r2_bc,
                in1=bbm[:, 0:16], op0=OP.mult, op1=OP.add)
            g_bf = wk.tile([128, 24], BF, tag="gbf")
            nc.scalar.activation(out=g_bf[:, 0:16], in_=gf[:, 0:16],
                                 func=ACT.Gelu)

            # ---- W2 GEMV: 24 passes x 2 chunks of 384; half B epilogue
            # (psD = segs 16..23) overlaps the first 16 waves ----
            psE = ps_m.tile([128, 384], DT, tag="mE")

            def w2_waves(lo, hi):
                for s in range(lo, hi):
                    st, sp = (s == 0), (s == 23)
                    lhs = g_bf[:, s:s + 1]
                    for g in range(2):
                        nc.tensor.matmul(
                            psE[32 * g:32 * g + 1, 0:384], lhs,
                            w2t[:, (s // 4) * 4 * E + (s % 4) * E + g * 384:
                                (s // 4) * 4 * E + (s % 4) * E + (g + 1) * 384],
                            start=st, stop=sp, tile_position=(0, 32 * g),
                            skip_group_check=True)

            w2_waves(0, 16)
            nc.vector.tensor_copy(out=m1r2[0:1, :], in_=psD[0:1, :])
            nc.scalar.copy(out=m1r2[32:33, :], in_=psD[32:33, :])
            for nt in range(4, 6):
                srcr, base = m1r2, 32 * (nt - 4)
                idr = ident[base:base + 1, base:base + 1]
                for c in range(4):
                    nc.tensor.transpose(
                        psS[:, 34 + 4 * nt + c:35 + 4 * nt + c],
                        srcr[base:base + 1, 128 * c:128 * c + 128], idr)
            nc.vector.tensor_mul(gf[:, 16:24], pv[:, 76:84], gps[:, 16:24])
            nc.vector.scalar_tensor_tensor(
                out=gf[:, 16:24], in0=gf[:, 16:24], scalar=r2_bc,
                in1=bbm[:, 16:24], op0=OP.mult, op1=OP.add)
            nc.scalar.activation(out=g_bf[:, 16:24], in_=gf[:, 16:24],
                                 func=ACT.Gelu)
            dummy(ACT.Sqrt, g_bf[0:1, 0:4])
            w2_waves(16, 24)
            warm(W_M3)
            m2r = wk.tile([34, 384], DT, tag="m2r")
            nc.vector.tensor_copy(out=m2r[0:1, :], in_=psE[0:1, :])
            nc.scalar.copy(out=m2r[32:33, :], in_=psE[32:33, :])
            pu2 = psS[:, 58:64]
            for r in range(2):
                idr = ident[32 * r:32 * r + 1, 32 * r:32 * r + 1]
                for c in range(3):
                    nc.tensor.transpose(
                        psS[:, 58 + 3 * r + c:59 + 3 * r + c],
                        m2r[32 * r:32 * r + 1, 128 * c:128 * c + 128], idr)
            warm(W_M4)
            # u'' = (u' + b2) + m2q*q2
            d6 = wk.tile([128, 6], DT, tag="d6")
            nc.vector.tensor_mul(d6[:], pv[:, 36:42], pu2)
            nc.vector.tensor_add(u_cm, upb[:], d6[:])
            nc.vector.tensor_copy(out=u_bf[:], in_=u_cm)
            stats_mm(u_cm)

        # ---- classifier (weights prefetched during layer 10) ----
        fcm, fb, qc2, wc1t, wc2 = clf

        stats_mm(u_cm)
        nc.tensor.matmul(psS[0:1, 0:12], onesc[:], stat_in[:],
                         start=True, stop=True)
        psC = ps_m.tile([128, 512], DT, tag="mC")
        psD = ps_m.tile([128, 512], DT, tag="mD")
        for s in range(6):
            st, sp = (s == 0), (s == 5)
            lhs = u_bf[:, s:s + 1]
            for nt in range(6):
                pt, row = (psC, nt) if nt < 4 else (psD, nt - 4)
                nc.tensor.matmul(
                    pt[32 * row:32 * row + 1, 0:512], lhs,
                    wc1t[:, s * HID + nt * 512: s * HID + nt * 512 + 512],
                    start=st, stop=sp, tile_position=(0, 32 * row),
                    skip_group_check=True)
        scal3f, sdf = stats_finish("lf")
        muf_bc, rf_bc, nmrf_bc = stats_bcast(scal3f, "lf")
        dummy(ACT.Gelu, sdf)
        bcmf = wk.tile([128, 24], DT, tag="bcmf")
        nc.vector.scalar_tensor_tensor(
            out=bcmf[:], in0=fcm[:, 24:48], scalar=nmrf_bc, in1=fcm[:, 48:72],
            op0=OP.mult, op1=OP.add)
        warm(6)
        m1r = wk.tile([66, 512], DT, tag="m1r")
        nc.vector.tensor_copy(out=m1r[0:1, :], in_=psC[0:1, :])
        nc.scalar.copy(out=m1r[32:33, :], in_=psC[32:33, :])
        nc.vector.tensor_copy(out=m1r[64:65, :], in_=psC[64:65, :])
        m1r2 = wk.tile([66, 512], DT, tag="m1r2")
        nc.scalar.copy(out=m1r2[64:65, :], in_=psC[96:97, :])
        nc.vector.tensor_copy(out=m1r2[0:1, :], in_=psD[0:1, :])
        nc.scalar.copy(out=m1r2[32:33, :], in_=psD[32:33, :])
        gps = psS[:, 34:58]
        for nt in range(6):
            if nt < 3:
                srcr, base = m1r, 32 * nt
            elif nt == 3:
                srcr, base = m1r2, 64
            else:
                srcr, base = m1r2, 32 * (nt - 4)
            idr = ident[base:base + 1, base:base + 1]
            for c in range(4):
                nc.tensor.transpose(
                    psS[:, 34 + 4 * nt + c:35 + 4 * nt + c],
                    srcr[base:base + 1, 128 * c:128 * c + 128], idr)
        warm(3)
        gf = wk.tile([128, 24], DT, tag="gf")
        nc.vector.tensor_mul(gf[:], gps, fcm[:, 0:24])
        nc.vector.scalar_tensor_tensor(
            out=gf[:], in0=gf[:], scalar=rf_bc, in1=bcmf[:],
            op0=OP.mult, op1=OP.add)
        gc_bf = wk.tile([128, 24], BF, tag="gbf")
        gelu_to(gc_bf, gf, [128, 24])

        psF = ps_m.tile([128, 512], DT, tag="mF")
        for s in range(24):
            st, sp = (s == 0), (s == 23)
            lhs = gc_bf[:, s:s + 1]
            wsrc = wc2[s // 3]
            sl = s % 3
            for g in range(2):
                nc.tensor.matmul(
                    psF[32 * g:32 * g + 1, 0:500], lhs,
                    wsrc[:, sl * CLS + g * 500: sl * CLS + (g + 1) * 500],
                    start=st, stop=sp, tile_position=(0, 32 * g),
                    skip_group_check=True)
        warm(2)
        lg = wk.tile([1, CLS], DT, tag="lg")
        nc.vector.tensor_copy(out=lg[0:1, 0:500], in_=psF[0:1, 0:500])
        nc.scalar.copy(out=lg[0:1, 500:1000], in_=psF[32:33, 0:500])
        nc.vector.tensor_mul(lg[:], lg[:], qc2[:])
        nc.vector.tensor_add(lg[:], lg[:], fb[:])

        # log_softmax
        mx = wk.tile([1, 1], DT, tag="mx")
        nc.vector.reduce_max(mx[:], lg[:], axis=AX.X)
        sh = wk.tile([1, CLS], DT, tag="sh")
        nc.vector.tensor_scalar(
            out=sh[:], in0=lg[:], scalar1=mx[:], scalar2=None, op0=OP.subtract)
        se = wk.tile([1, 1], DT, tag="se")
        nc.scalar.activation(out=lg[:], in_=sh[:], func=ACT.Exp, accum_out=se[:])
        lse = wk.tile([1, 1], DT, tag="lse")
        nc.scalar.activation(out=lse[:], in_=se[:], func=ACT.Ln)
        nc.vector.tensor_scalar(
            out=sh[:], in0=sh[:], scalar1=lse[:], scalar2=None, op0=OP.subtract)
        nc.sync.dma_start(out=out_t[:, :], in_=sh[:])

    nc.compile()
    return nc


def _cm(v, nseg):
    """flat [-1] -> [128, nseg] with cm[p, s] = v[128s + p]"""
    return np.ascontiguousarray(np.asarray(v, np.float32).reshape(nseg, 128).T)


def _q8(w):
    """[K, N] f32 -> (fp8 e3m4 col-scaled, scales[N])"""
    w = np.asarray(w, np.float32)
    s = np.abs(w).max(axis=0) / F8LIM
    s = np.where(s == 0, 1.0, s)
    return (w / s).astype(ml_dtypes.float8_e3m4), s


def prep_inputs(inputs):
    f32 = lambda x: np.ascontiguousarray(np.asarray(x, dtype=np.float32))
    m = {}
    Wv, Wt = inputs["Wv"], inputs["Wtheta"]
    W1, W2 = inputs["W1"], inputs["W2"]
    for l in range(L):
        s1l = np.asarray(inputs["ln1_s"][l], np.float32)
        b1l = np.asarray(inputs["ln1_b"][l], np.float32)
        s2l = np.asarray(inputs["ln2_s"][l], np.float32)
        b2l = np.asarray(inputs["ln2_b"][l], np.float32)
        wv = np.asarray(Wv[l], np.float32) * s1l[:, None]    # diag(s)Wv
        wt = np.asarray(Wt[l], np.float32) * s1l[:, None]
        cv = wv.sum(axis=0)
        ct = wt.sum(axis=0)
        bv = b1l @ np.asarray(Wv[l], np.float32)
        bt = b1l @ np.asarray(Wt[l], np.float32)
        wvq, sv = _q8(wv)
        wtq, st = _q8(wt)
        wv3 = wvq.reshape(6, 128, 2, 384)
        wt3 = wtq.reshape(6, 128, 2, 384)
        blk = np.concatenate([wt3, wv3], axis=2)     # [6,128,4,384] t|t|a|a
        m[f"wvt{l}"] = np.ascontiguousarray(
            blk.transpose(1, 0, 2, 3).reshape(128, 6 * 1536))
        w1 = np.asarray(W1[l], np.float32) * s2l[:, None]    # diag(s2)W1
        c1 = w1.sum(axis=0)
        bb1 = b2l @ np.asarray(W1[l], np.float32) + np.asarray(
            inputs["b1"][l], np.float32)
        w1q, q1 = _q8(w1)
        m[f"w1t{l}"] = np.ascontiguousarray(
            w1q.reshape(6, 128, HID).transpose(1, 0, 2).reshape(128, 6 * HID))
        w2q, q2 = _q8(W2[l])
        m[f"w2t{l}"] = np.ascontiguousarray(
            w2q.reshape(6, 4, 128, E).transpose(2, 0, 1, 3).reshape(
                128, 6 * 4 * E))
        pv = np.concatenate([
            _cm(sv, 6), _cm(st, 6), _cm(cv, 6), _cm(ct, 6),
            _cm(bv, 6), _cm(bt, 6),
            _cm(q2, 6), _cm(inputs["b2"][l], 6),
            _cm(s1l, 6), _cm(b1l, 6),
            _cm(q1, 24), _cm(c1, 24), _cm(bb1, 24)], axis=1)
        m[f"pv{l}"] = f32(pv)
    sfl = np.asarray(inputs["lnf_s"], np.float32)
    bfl = np.asarray(inputs["lnf_b"], np.float32)
    wc1 = np.asarray(inputs["Wc1"], np.float32) * sfl[:, None]
    ccl = wc1.sum(axis=0)
    bcl = bfl @ np.asarray(inputs["Wc1"], np.float32) + np.asarray(
        inputs["bc1"], np.float32)
    wc1q, qc1 = _q8(wc1)
    m["wc1t"] = np.ascontiguousarray(
        wc1q.reshape(6, 128, HID).transpose(1, 0, 2).reshape(128, 6 * HID))
    wc2q, sc2 = _q8(inputs["Wc2"])
    wc2q = wc2q.reshape(24, 128, CLS).transpose(1, 0, 2)
    for c in range(8):
        m[f"wc2{c}"] = np.ascontiguousarray(
            wc2q[:, 3 * c:3 * c + 3].reshape(128, 3 * CLS))
    m["fcm"] = f32(np.concatenate([
        _cm(qc1, 24), _cm(ccl, 24), _cm(bcl, 24)], axis=1))
    m["fb"] = f32(np.asarray(inputs["bc2"]).reshape(1, CLS))
    m["qc2"] = f32(sc2.reshape(1, CLS))
    m["identf"] = np.eye(128, dtype=np.float32)
    m["onesc"] = np.ones((128, 1), np.float32)
    m["onesr"] = np.ones((1, 128), np.float32)
    u0 = np.asarray(inputs["class_token"]).reshape(E) + \
        np.asarray(inputs["pos"]).reshape(-1, E)[-1]
    m["u0"] = _cm(u0, 6)
    return m


_CACHED = {}


def kernel(**inputs) -> np.ndarray:
    b = int(np.asarray(inputs["x"]).shape[0])
    in_map = prep_inputs(inputs)
    if "nc" not in _CACHED:
        _CACHED["nc"] = build_program()
    nc = _CACHED["nc"]
    r = run_bass_kernel_spmd(nc, [in_map], core_ids=[0])
    out = np.asarray(r.results[0]["out"]).reshape(1, CLS)
    return np.ascontiguousarray(np.broadcast_to(out, (b, CLS)).astype(np.float32))
